# revision 2
# baseline (speedup 1.0000x reference)
"""Trainium2 Bass kernel for DeformRoIPooling (DCNv2 deform_psroi_pooling).

Strategy:
  - Host precomputes, per ROI, the set of feature-map pixels touched
    (bilinear 4-neighborhoods of all valid samples) and a dense weight
    matrix W [support, 49] that folds bilinear weights, valid mask and
    1/cnt. out[bin, c] = sum_slot W[slot, bin] * x_nhwc[pix[slot], c].
  - Sharding: image b -> cores {2b, 2b+1}; each core processes ~32 ROIs
    of its image (balanced by K-tile count). SPMD: one program, per-core
    data. ROIs are sorted by size so slot j has the same K-tile count on
    every core (max across cores, zero-padded).
  - Device: chunked dma_gather (pixels -> partitions, channels on the
    free axis) + TensorE matmul (W as stationary operand) accumulating
    [49, 256] per ROI in PSUM, DVE copy to SBUF, DMA out.
"""
import numpy as np

SPATIAL_SCALE = 0.0625
POOLED = 7
PART = 7
SAMPLE = 4
TRANS_STD = 0.1
H = W = 96
C = 256
B = 4
P, S = POOLED, SAMPLE
NBIN = P * P
N_CORES = 8
CHUNK_TILES = 16


# ----------------------------------------------------------------------------
# Host-side precompute (float32, mirrors the reference expression tree)
# ----------------------------------------------------------------------------

def _sample_weights(rois, offset):
    f = np.float32
    rois = rois.astype(f)
    offset = offset.astype(f)
    N = rois.shape[0]
    bidx = rois[:, 0].astype(np.int32)
    roi_start_w = np.round(rois[:, 1]) * f(SPATIAL_SCALE) - f(0.5)
    roi_start_h = np.round(rois[:, 2]) * f(SPATIAL_SCALE) - f(0.5)
    roi_end_w = np.round(rois[:, 3] + f(1.0)) * f(SPATIAL_SCALE) - f(0.5)
    roi_end_h = np.round(rois[:, 4] + f(1.0)) * f(SPATIAL_SCALE) - f(0.5)
    roi_w = np.maximum(roi_end_w - roi_start_w, f(0.1))
    roi_h = np.maximum(roi_end_h - roi_start_h, f(0.1))
    bin_w = roi_w / f(P)
    bin_h = roi_h / f(P)
    sub_w = bin_w / f(S)
    sub_h = bin_h / f(S)
    ph = np.arange(P)
    pw = np.arange(P)
    part_h = np.floor(ph.astype(f) / f(P) * f(PART)).astype(np.int32)
    part_w = np.floor(pw.astype(f) / f(P) * f(PART)).astype(np.int32)
    tx = offset[:, 0][:, part_h[:, None], part_w[None, :]] * f(TRANS_STD)
    ty = offset[:, 1][:, part_h[:, None], part_w[None, :]] * f(TRANS_STD)
    wstart = (pw[None, None, :].astype(f) * bin_w[:, None, None]
              + roi_start_w[:, None, None] + tx * roi_w[:, None, None])
    hstart = (ph[None, :, None].astype(f) * bin_h[:, None, None]
              + roi_start_h[:, None, None] + ty * roi_h[:, None, None])
    samp = np.arange(S).astype(f)
    ws = wstart[..., None, None] + samp[None, None, None, None, :] * sub_w[:, None, None, None, None]
    hs = hstart[..., None, None] + samp[None, None, None, :, None] * sub_h[:, None, None, None, None]
    valid = (ws > f(-0.5)) & (ws < f(W - 0.5)) & (hs > f(-0.5)) & (hs < f(H - 0.5))
    wc = np.clip(ws, f(0.0), f(W - 1.0))
    hc = np.clip(hs, f(0.0), f(H - 1.0))
    x0 = np.floor(wc).astype(np.int32)
    x1 = np.ceil(wc).astype(np.int32)
    y0 = np.floor(hc).astype(np.int32)
    y1 = np.ceil(hc).astype(np.int32)
    dx = wc - x0.astype(f)
    dy = hc - y0.astype(f)
    one = f(1.0)
    w00 = (one - dx) * (one - dy)
    w10 = (one - dx) * dy
    w01 = dx * (one - dy)
    w11 = dx * dy
    cnt = valid.sum(axis=(3, 4)).astype(f)
    inv_cnt = np.where(cnt > 0, one / np.maximum(cnt, one), f(0.0))
    vf = valid.astype(f)
    wall = np.stack([w00, w10, w01, w11], axis=-1) * vf[..., None]
    wall = wall * inv_cnt[:, :, :, None, None, None]
    pixall = np.stack([y0 * W + x0, y1 * W + x0, y0 * W + x1, y1 * W + x1], axis=-1)
    N = rois.shape[0]
    return (bidx, pixall.reshape(N, NBIN, S * S * 4),
            wall.reshape(N, NBIN, S * S * 4).astype(np.float32))


def _roi_tables(pix_n, wgt_n):
    pixf = pix_n.reshape(-1)
    wf = wgt_n.reshape(-1).astype(np.float64)
    binf = np.repeat(np.arange(NBIN), S * S * 4)
    nz = wf != 0.0
    pixf, wf, binf = pixf[nz], wf[nz], binf[nz]
    if pixf.size == 0:
        return np.zeros(1, np.int32), np.zeros((1, NBIN), np.float64)
    support, inv = np.unique(pixf, return_inverse=True)
    Wmat = np.zeros((support.size, NBIN), np.float64)
    np.add.at(Wmat, (inv, binf), wf)
    return support.astype(np.int32), Wmat


def _build_core_tables(x, rois, offset):
    N = rois.shape[0]
    bidx, pix, wgt = _sample_weights(rois, offset)
    supports, wmats = [], []
    for n in range(N):
        s, w = _roi_tables(pix[n], wgt[n])
        supports.append(s)
        wmats.append(w)
    ktiles = np.array([(len(s) + 127) // 128 for s in supports])

    core_rois = [[] for _ in range(N_CORES)]
    core_load = [0] * N_CORES
    cores_per_img = N_CORES // B
    for b in range(B):
        cand = list(range(b * cores_per_img, (b + 1) * cores_per_img))
        ids = np.where(bidx == b)[0]
        ids = ids[np.argsort(-ktiles[ids], kind="stable")]
        for n in ids:
            c = min(cand, key=lambda cc: (core_load[cc], len(core_rois[cc])))
            core_rois[c].append(int(n))
            core_load[c] += int(ktiles[n])
    for c in range(N_CORES):
        core_rois[c].sort(key=lambda n: -int(ktiles[n]))
    n_slots = max(1, max(len(r) for r in core_rois))
    K = np.zeros(n_slots, np.int64)
    for c in range(N_CORES):
        for j, n in enumerate(core_rois[c]):
            K[j] = max(K[j], ktiles[n])
    K = np.maximum(K, 1)
    tile_off = np.concatenate([[0], np.cumsum(K)]).astype(np.int64)
    total_tiles = int(tile_off[-1])

    idx_all = np.zeros((N_CORES, total_tiles * 128), np.int16)
    w_all = np.zeros((N_CORES, total_tiles * 128, NBIN), np.float32)
    roi_of_slot = np.full((N_CORES, n_slots), -1, np.int64)
    for c in range(N_CORES):
        for j, n in enumerate(core_rois[c]):
            s, wm = supports[n], wmats[n]
            o = int(tile_off[j]) * 128
            idx_all[c, o:o + len(s)] = s
            w_all[c, o:o + len(s)] = wm.astype(np.float32)
            roi_of_slot[c, j] = n
    # dma_gather index layout: index i -> partition i%16, col i//16,
    # replicated 8x to fill 128 partitions (one copy per Q7 core)
    idx_wrapped = idx_all.reshape(N_CORES, total_tiles * 8, 16).transpose(0, 2, 1)
    idx_sb = np.tile(idx_wrapped, (1, 8, 1))
    # stationary-operand layout: w_sb[p, t, m] = w_all[t*128 + p, m]
    w_sb = w_all.reshape(N_CORES, total_tiles, 128, NBIN).transpose(0, 2, 1, 3)
    xt = np.ascontiguousarray(x.transpose(0, 2, 3, 1).reshape(B, H * W, C))
    xt_core = np.stack([xt[b] for b in range(B) for _ in range(cores_per_img)])
    return dict(
        n_slots=n_slots, K=K, tile_off=tile_off, total_tiles=total_tiles,
        idx_sb=np.ascontiguousarray(idx_sb),
        w_sb=np.ascontiguousarray(w_sb),
        xt_core=xt_core, roi_of_slot=roi_of_slot,
    )


def _build_chunks(tile_off, n_slots):
    """Pack consecutive ROI slots into gather chunks of <= CHUNK_TILES K-tiles."""
    chunks = []
    s0 = 0
    while s0 < n_slots:
        s1 = s0
        while (s1 < n_slots
               and tile_off[s1 + 1] - tile_off[s0] <= CHUNK_TILES):
            s1 += 1
        if s1 == s0:   # single ROI larger than CHUNK_TILES
            s1 = s0 + 1
        chunks.append((s0, s1, int(tile_off[s0]), int(tile_off[s1])))
        s0 = s1
    return chunks


# ----------------------------------------------------------------------------
# Device program
# ----------------------------------------------------------------------------

_NC_CACHE = {}


def _build_nc(n_slots, tile_off, total_tiles, chunks):
    import concourse.bacc as bacc
    import concourse.mybir as mybir
    from concourse import tile
    from concourse.library_config import mlp

    nc = bacc.Bacc("TRN2", target_bir_lowering=False, debug=False)
    f32 = mybir.dt.float32
    xt_d = nc.dram_tensor("xt", [H * W, C], f32, kind="ExternalInput")
    idx_d = nc.dram_tensor("idx", [128, total_tiles * 8], mybir.dt.int16,
                           kind="ExternalInput")
    w_d = nc.dram_tensor("w", [128, total_tiles, NBIN], f32, kind="ExternalInput")
    out_d = nc.dram_tensor("out", [n_slots, NBIN, C], f32, kind="ExternalOutput")

    max_ct = max(t1 - t0 for _, _, t0, t1 in chunks)
    with tile.TileContext(nc) as tc:
        with (
            tc.tile_pool(name="const", bufs=1) as cpool,
            tc.tile_pool(name="g", bufs=3) as gpool,
            tc.tile_pool(name="wp", bufs=3) as wpool,
            tc.tile_pool(name="op", bufs=4) as opool,
            tc.tile_pool(name="ps", bufs=6, space="PSUM") as ppool,
        ):
            nc.gpsimd.load_library(mlp)
            idx_sb = cpool.tile([128, total_tiles * 8], mybir.dt.int16)
            nc.sync.dma_start(idx_sb[:], idx_d[:])
            for (s0, s1, t0, t1) in chunks:
                ct = t1 - t0
                g = gpool.tile([128, max_ct, C], f32, tag="g")
                nc.gpsimd.dma_gather(
                    g[:, :ct, :], xt_d[:], idx_sb[:, t0 * 8:t1 * 8],
                    ct * 128, ct * 128, C,
                    # single_packet coalesces an engine's whole stream into one
                    # packet; >16 descs/engine (256 idxs) exceeds the 16KB
                    # packet limit and wedges the SDMA engine.
                    single_packet=False,
                )
                wt = wpool.tile([128, max_ct, NBIN], f32, tag="w")
                nc.sync.dma_start(wt[:, :ct, :], w_d[:, t0:t1, :])
                for j in range(s0, s1):
                    k0 = int(tile_off[j]) - t0
                    k1 = int(tile_off[j + 1]) - t0
                    ps = ppool.tile([NBIN, C], f32, tag="p")
                    for t in range(k0, k1):
                        nc.tensor.matmul(
                            ps[:, :], wt[:, t, :], g[:, t, :],
                            start=(t == k0), stop=(t == k1 - 1),
                        )
                    o = opool.tile([NBIN, C], f32, tag="o")
                    nc.vector.tensor_copy(o[:], ps[:])
                    nc.sync.dma_start(out_d[j], o[:])
    nc.compile()
    return nc


def build_program(x, rois, offset):
    """Host tables + (cached) compiled bass program. Returns (tables, nc)."""
    t = _build_core_tables(x, rois, offset)
    chunks = _build_chunks(t["tile_off"], t["n_slots"])
    key = (t["n_slots"], tuple(int(k) for k in t["K"]))
    nc = _NC_CACHE.get(key)
    if nc is None:
        nc = _build_nc(t["n_slots"], t["tile_off"], t["total_tiles"], chunks)
        _NC_CACHE[key] = nc
    return t, nc


def kernel(x, rois, offset):
    from concourse.bass_utils import run_bass_kernel_spmd

    x = np.ascontiguousarray(np.asarray(x, dtype=np.float32))
    rois = np.asarray(rois, dtype=np.float32)
    offset = np.asarray(offset, dtype=np.float32)
    N = rois.shape[0]

    t, nc = build_program(x, rois, offset)
    in_maps = [
        dict(
            xt=t["xt_core"][c],
            idx=t["idx_sb"][c],
            w=t["w_sb"][c],
        )
        for c in range(N_CORES)
    ]
    res = run_bass_kernel_spmd(nc, in_maps, core_ids=list(range(N_CORES)))
    out = np.zeros((N, C, P, P), np.float32)
    for c in range(N_CORES):
        co = res.results[c]["out"]
        for j in range(t["n_slots"]):
            n = int(t["roi_of_slot"][c, j])
            if n >= 0:
                out[n] = co[j].T.reshape(C, P, P)
    return out


# revision 7
# speedup vs baseline: 1.2116x; 1.2116x over previous
"""Trainium2 Bass kernel for DeformRoIPooling (DCNv2 deform_psroi_pooling).

Strategy:
  - Host precomputes, per ROI, the set of feature-map pixels touched
    (bilinear 4-neighborhoods of all valid samples) and a dense weight
    matrix W [support, 49] that folds bilinear weights, valid mask and
    1/cnt. out[bin, c] = sum_slot W[slot, bin] * x_nhwc[pix[slot], c].
  - Sharding: image b -> cores {2b, 2b+1}; each core processes ~32 ROIs
    of its image (balanced by K-tile count). SPMD: one program, per-core
    data. ROIs are sorted by size so slot j has the same K-tile count on
    every core (max across cores, zero-padded).
  - Device: chunked dma_gather (pixels -> partitions, channels on the
    free axis) + TensorE matmul (W as stationary operand) accumulating
    [49, 256] per ROI in PSUM, DVE copy to SBUF, DMA out.
"""
import numpy as np

SPATIAL_SCALE = 0.0625
POOLED = 7
PART = 7
SAMPLE = 4
TRANS_STD = 0.1
H = W = 96
C = 256
B = 4
P, S = POOLED, SAMPLE
NBIN = P * P
N_CORES = 8
GRP = 4               # pixels per gather element (x-aligned group, 4KB)
CHUNK_TILES = 6


# ----------------------------------------------------------------------------
# Host-side precompute (float32, mirrors the reference expression tree)
# ----------------------------------------------------------------------------

def _sample_weights(rois, offset):
    f = np.float32
    rois = rois.astype(f)
    offset = offset.astype(f)
    N = rois.shape[0]
    bidx = rois[:, 0].astype(np.int32)
    roi_start_w = np.round(rois[:, 1]) * f(SPATIAL_SCALE) - f(0.5)
    roi_start_h = np.round(rois[:, 2]) * f(SPATIAL_SCALE) - f(0.5)
    roi_end_w = np.round(rois[:, 3] + f(1.0)) * f(SPATIAL_SCALE) - f(0.5)
    roi_end_h = np.round(rois[:, 4] + f(1.0)) * f(SPATIAL_SCALE) - f(0.5)
    roi_w = np.maximum(roi_end_w - roi_start_w, f(0.1))
    roi_h = np.maximum(roi_end_h - roi_start_h, f(0.1))
    bin_w = roi_w / f(P)
    bin_h = roi_h / f(P)
    sub_w = bin_w / f(S)
    sub_h = bin_h / f(S)
    ph = np.arange(P)
    pw = np.arange(P)
    part_h = np.floor(ph.astype(f) / f(P) * f(PART)).astype(np.int32)
    part_w = np.floor(pw.astype(f) / f(P) * f(PART)).astype(np.int32)
    tx = offset[:, 0][:, part_h[:, None], part_w[None, :]] * f(TRANS_STD)
    ty = offset[:, 1][:, part_h[:, None], part_w[None, :]] * f(TRANS_STD)
    wstart = (pw[None, None, :].astype(f) * bin_w[:, None, None]
              + roi_start_w[:, None, None] + tx * roi_w[:, None, None])
    hstart = (ph[None, :, None].astype(f) * bin_h[:, None, None]
              + roi_start_h[:, None, None] + ty * roi_h[:, None, None])
    samp = np.arange(S).astype(f)
    ws = wstart[..., None, None] + samp[None, None, None, None, :] * sub_w[:, None, None, None, None]
    hs = hstart[..., None, None] + samp[None, None, None, :, None] * sub_h[:, None, None, None, None]
    valid = (ws > f(-0.5)) & (ws < f(W - 0.5)) & (hs > f(-0.5)) & (hs < f(H - 0.5))
    wc = np.clip(ws, f(0.0), f(W - 1.0))
    hc = np.clip(hs, f(0.0), f(H - 1.0))
    x0 = np.floor(wc).astype(np.int32)
    x1 = np.ceil(wc).astype(np.int32)
    y0 = np.floor(hc).astype(np.int32)
    y1 = np.ceil(hc).astype(np.int32)
    dx = wc - x0.astype(f)
    dy = hc - y0.astype(f)
    one = f(1.0)
    w00 = (one - dx) * (one - dy)
    w10 = (one - dx) * dy
    w01 = dx * (one - dy)
    w11 = dx * dy
    cnt = valid.sum(axis=(3, 4)).astype(f)
    inv_cnt = np.where(cnt > 0, one / np.maximum(cnt, one), f(0.0))
    vf = valid.astype(f)
    wall = np.stack([w00, w10, w01, w11], axis=-1) * vf[..., None]
    wall = wall * inv_cnt[:, :, :, None, None, None]
    pixall = np.stack([y0 * W + x0, y1 * W + x0, y0 * W + x1, y1 * W + x1], axis=-1)
    N = rois.shape[0]
    return (bidx, pixall.reshape(N, NBIN, S * S * 4),
            wall.reshape(N, NBIN, S * S * 4).astype(np.float32))


def _roi_tables(pix_n, wgt_n):
    """Dedup to 4-pixel groups (x-aligned). Returns (groups [M], W [M,4,49])."""
    pixf = pix_n.reshape(-1)
    wf = wgt_n.reshape(-1).astype(np.float64)
    binf = np.repeat(np.arange(NBIN), S * S * 4)
    nz = wf != 0.0
    pixf, wf, binf = pixf[nz], wf[nz], binf[nz]
    if pixf.size == 0:
        return np.zeros(1, np.int32), np.zeros((1, GRP, NBIN), np.float64)
    support, inv = np.unique(pixf // GRP, return_inverse=True)
    Wmat = np.zeros((support.size, GRP, NBIN), np.float64)
    np.add.at(Wmat, (inv, pixf % GRP, binf), wf)
    return support.astype(np.int32), Wmat


def _build_core_tables(x, rois, offset):
    N = rois.shape[0]
    bidx, pix, wgt = _sample_weights(rois, offset)
    supports, wmats = [], []
    for n in range(N):
        s, w = _roi_tables(pix[n], wgt[n])
        supports.append(s)
        wmats.append(w)
    ktiles = np.array([(len(s) + 127) // 128 for s in supports])

    core_rois = [[] for _ in range(N_CORES)]
    core_load = [0] * N_CORES
    cores_per_img = N_CORES // B
    for b in range(B):
        cand = list(range(b * cores_per_img, (b + 1) * cores_per_img))
        ids = np.where(bidx == b)[0]
        ids = ids[np.argsort(-ktiles[ids], kind="stable")]
        for n in ids:
            c = min(cand, key=lambda cc: (core_load[cc], len(core_rois[cc])))
            core_rois[c].append(int(n))
            core_load[c] += int(ktiles[n])
    for c in range(N_CORES):
        core_rois[c].sort(key=lambda n: -int(ktiles[n]))
    n_slots = max(1, max(len(r) for r in core_rois))
    K = np.zeros(n_slots, np.int64)
    for c in range(N_CORES):
        for j, n in enumerate(core_rois[c]):
            K[j] = max(K[j], ktiles[n])
    K = np.maximum(K, 1)
    tile_off = np.concatenate([[0], np.cumsum(K)]).astype(np.int64)
    total_tiles = int(tile_off[-1])

    idx_all = np.zeros((N_CORES, total_tiles * 128), np.int16)
    w_all = np.zeros((N_CORES, total_tiles * 128, GRP, NBIN), np.float32)
    roi_of_slot = np.full((N_CORES, n_slots), -1, np.int64)
    for c in range(N_CORES):
        for j, n in enumerate(core_rois[c]):
            s, wm = supports[n], wmats[n]
            o = int(tile_off[j]) * 128
            idx_all[c, o:o + len(s)] = s
            w_all[c, o:o + len(s)] = wm.astype(np.float32)
            roi_of_slot[c, j] = n
    # dma_gather index layout: index i -> partition i%16, col i//16,
    # replicated 8x to fill 128 partitions (one copy per Q7 core)
    idx_wrapped = idx_all.reshape(N_CORES, total_tiles * 8, 16).transpose(0, 2, 1)
    idx_sb = np.tile(idx_wrapped, (1, 8, 1))
    # stationary-operand layout: w_sb[p, t, j, m] = w_all[t*128 + p, j, m]
    w_sb = w_all.reshape(N_CORES, total_tiles, 128, GRP, NBIN).transpose(0, 2, 1, 3, 4)
    xt = np.ascontiguousarray(x.transpose(0, 2, 3, 1).reshape(B, H * W, C))
    xt_core = np.stack([xt[b] for b in range(B) for _ in range(cores_per_img)])
    return dict(
        n_slots=n_slots, K=K, tile_off=tile_off, total_tiles=total_tiles,
        idx_sb=np.ascontiguousarray(idx_sb),
        w_sb=np.ascontiguousarray(w_sb),
        xt_core=xt_core, roi_of_slot=roi_of_slot,
    )


def _build_chunks(tile_off, n_slots):
    """Pack consecutive ROI slots into gather chunks of <= CHUNK_TILES K-tiles."""
    chunks = []
    s0 = 0
    while s0 < n_slots:
        s1 = s0
        while (s1 < n_slots
               and tile_off[s1 + 1] - tile_off[s0] <= CHUNK_TILES):
            s1 += 1
        if s1 == s0:   # single ROI larger than CHUNK_TILES
            s1 = s0 + 1
        chunks.append((s0, s1, int(tile_off[s0]), int(tile_off[s1])))
        s0 = s1
    return chunks


# ----------------------------------------------------------------------------
# Device program
# ----------------------------------------------------------------------------

_NC_CACHE = {}


def _build_nc(n_slots, tile_off, total_tiles, chunks):
    import concourse.bacc as bacc
    import concourse.mybir as mybir
    from concourse import tile
    from concourse.library_config import mlp

    nc = bacc.Bacc("TRN2", target_bir_lowering=False, debug=False)
    f32 = mybir.dt.float32
    EL = GRP * C  # 1024 f32 per gathered element (4 pixels x 256 ch)
    xt_d = nc.dram_tensor("xt", [H * W // GRP, EL], f32, kind="ExternalInput")
    idx_d = nc.dram_tensor("idx", [128, total_tiles * 8], mybir.dt.int16,
                           kind="ExternalInput")
    w_d = nc.dram_tensor("w", [128, total_tiles, GRP, NBIN], f32,
                         kind="ExternalInput")
    out_d = nc.dram_tensor("out", [n_slots, NBIN, C], f32, kind="ExternalOutput")

    max_ct = max(t1 - t0 for _, _, t0, t1 in chunks)
    with tile.TileContext(nc) as tc:
        with (
            tc.tile_pool(name="const", bufs=1) as cpool,
            tc.tile_pool(name="g", bufs=3) as gpool,
            tc.tile_pool(name="wp", bufs=3) as wpool,
            tc.tile_pool(name="op", bufs=4) as opool,
            tc.tile_pool(name="ps", bufs=6, space="PSUM") as ppool,
        ):
            nc.gpsimd.load_library(mlp)
            idx_sb = cpool.tile([128, total_tiles * 8], mybir.dt.int16)
            nc.sync.dma_start(idx_sb[:], idx_d[:])
            for (s0, s1, t0, t1) in chunks:
                ct = t1 - t0
                g = gpool.tile([128, max_ct, EL], f32, tag="g")
                nc.gpsimd.dma_gather(
                    g[:, :ct, :], xt_d[:], idx_sb[:, t0 * 8:t1 * 8],
                    ct * 128, ct * 128, EL,
                    # single_packet coalesces an engine's whole stream into one
                    # packet; >16KB of descriptors per engine wedges the SDMA.
                    single_packet=False,
                )
                wt = wpool.tile([128, max_ct, GRP, NBIN], f32, tag="w")
                nc.sync.dma_start(wt[:, :ct, :, :], w_d[:, t0:t1, :, :])
                for j in range(s0, s1):
                    k0 = int(tile_off[j]) - t0
                    k1 = int(tile_off[j + 1]) - t0
                    ps = ppool.tile([NBIN, C], f32, tag="p")
                    for t in range(k0, k1):
                        for sub in range(GRP):
                            nc.tensor.matmul(
                                ps[:, :], wt[:, t, sub, :],
                                g[:, t, sub * C:(sub + 1) * C],
                                start=(t == k0 and sub == 0),
                                stop=(t == k1 - 1 and sub == GRP - 1),
                            )
                    o = opool.tile([NBIN, C], f32, tag="o")
                    nc.vector.tensor_copy(o[:], ps[:])
                    nc.sync.dma_start(out_d[j], o[:])
    nc.compile()
    return nc


def build_program(x, rois, offset):
    """Host tables + (cached) compiled bass program. Returns (tables, nc)."""
    t = _build_core_tables(x, rois, offset)
    chunks = _build_chunks(t["tile_off"], t["n_slots"])
    key = (t["n_slots"], tuple(int(k) for k in t["K"]))
    nc = _NC_CACHE.get(key)
    if nc is None:
        nc = _build_nc(t["n_slots"], t["tile_off"], t["total_tiles"], chunks)
        _NC_CACHE[key] = nc
    return t, nc


def kernel(x, rois, offset):
    from concourse.bass_utils import run_bass_kernel_spmd

    x = np.ascontiguousarray(np.asarray(x, dtype=np.float32))
    rois = np.asarray(rois, dtype=np.float32)
    offset = np.asarray(offset, dtype=np.float32)
    N = rois.shape[0]

    t, nc = build_program(x, rois, offset)
    in_maps = [
        dict(
            xt=t["xt_core"][c].reshape(H * W // GRP, GRP * C),
            idx=t["idx_sb"][c],
            w=t["w_sb"][c],
        )
        for c in range(N_CORES)
    ]
    res = run_bass_kernel_spmd(nc, in_maps, core_ids=list(range(N_CORES)))
    out = np.zeros((N, C, P, P), np.float32)
    for c in range(N_CORES):
        co = res.results[c]["out"]
        for j in range(t["n_slots"]):
            n = int(t["roi_of_slot"][c, j])
            if n >= 0:
                out[n] = co[j].T.reshape(C, P, P)
    return out
